# revision 11
# baseline (speedup 1.0000x reference)
"""Multi-head attention Trainium2 kernel, 8-way sharded, mask-compacted.

Problem: x[4,2048,1024] -> qkv proj (w_qkv [3072,1024]) -> 16-head attention
with key-padding mask -> tail proj (w_tail [1024,1024]) + b_tail.

Sharding: 8 shards = 4 batches x 2 head-groups (8 heads each). Each core
computes, for its (batch b, head-group hg):
  - q projection of x[b] for its 8 heads (all 2048 tokens)
  - k/v projections of the MASK-KEPT tokens only (host-compacted; masked
    keys contribute exactly zero to the softmax, so dropping them up front
    is exact and halves the attention work)
  - [kept x 2048] masked attention per head
  - partial tail matmul y_part = attn_cat @ w_tail[:, cat_slice].T
Host unshards: out[b] = y_part[2b] + y_part[2b+1] + b_tail.  No collectives.

All matmul operands are bf16 (PE 1 cyc/row); PSUM accumulation is f32.
Softmax denominator comes from a ones-column appended to V; the per-token
reciprocal is broadcast across cat-partitions via gpsimd partition_broadcast
(no PE transposes). q/k projections of pair j+1 are interleaved into pair
j's attention stream to keep the PE dense (p-state) while ACT runs exp.
"""

import time as _time

import numpy as np
from contextlib import ExitStack

import concourse.bass as bass
import concourse.mybir as mybir
import concourse.tile as tile
from concourse.bass_utils import run_bass_kernel_spmd

# ---------------------------------------------------------------------------
# walrus in this env accepts at most 2 sync waits per instruction; Tile's
# scheduler emits up to 10. Post-pass: peel excess waits onto same-engine
# NoOps inserted immediately before the offending instruction (same engine
# stream position => identical synchronization semantics).
MAX_WAITS = 1


def split_excess_waits(nc):
    for fn in nc.m.functions:
        for bb in fn.blocks:
            insts = list(bb.instructions)
            out = []
            changed = False
            for inst in insts:
                si = inst.sync_info
                waits = list(si.on_wait) if si is not None else []
                if len(waits) > MAX_WAITS:
                    extra = waits[:-MAX_WAITS]
                    for ci in range(0, len(extra), MAX_WAITS):
                        chunk = extra[ci:ci + MAX_WAITS]
                        nop = mybir.InstNoOp(
                            name=f"{inst.name}-ws{ci}", ins=[], outs=[])
                        nop.engine = inst.engine
                        nop.sync_info = mybir.SyncInfo(
                            on_wait=chunk, on_update=[])
                        out.append(nop)
                    inst.sync_info = mybir.SyncInfo(
                        on_wait=waits[-MAX_WAITS:],
                        on_update=list(si.on_update))
                    changed = True
                out.append(inst)
            if changed:
                bb.instructions = out
# ---------------------------------------------------------------------------

D_MODEL = 1024
N_HEAD = 16
D_HEAD = 64
BN, T = 4, 2048
HPC = 8                      # heads per core
NPAIR = HPC // 2             # head pairs (2 heads share a 128-row tile)
CAT = HPC * D_HEAD           # 512 per-core tail contraction
QH = T // 2                  # q processed in two halves of 1024
KC = D_MODEL // 128          # 8 contraction chunks
F32 = mybir.dt.float32
BF16 = mybir.dt.bfloat16
LAG = 4                      # ST->AV software pipeline depth (key blocks)
NEG = -30000.0               # additive bias for padded keys: exp -> 0


def build_nc(tkp, split_waits=True):
    """tkp: padded kept-key count (multiple of 128)."""
    nkbk = tkp // 128        # kept-key blocks
    nc = bass.Bass()
    xT = nc.declare_dram_parameter("xT", [D_MODEL, T], BF16, isOutput=False)
    xkT = nc.declare_dram_parameter("xkT", [D_MODEL, tkp], BF16, isOutput=False)
    wqT = nc.declare_dram_parameter("wqT", [D_MODEL, CAT], BF16, isOutput=False)
    wkT = nc.declare_dram_parameter("wkT", [D_MODEL, CAT], BF16, isOutput=False)
    wvT = nc.declare_dram_parameter("wvT", [D_MODEL, CAT], BF16, isOutput=False)
    wtailT = nc.declare_dram_parameter("wtailT", [CAT, D_MODEL], BF16, isOutput=False)
    maskb_d = nc.declare_dram_parameter("maskb", [tkp], F32, isOutput=False)
    ones8 = nc.declare_dram_parameter("ones8", [128, HPC], BF16, isOutput=False)
    y = nc.declare_dram_parameter("y", [T, D_MODEL], F32, isOutput=True)

    with ExitStack() as ctx:
        tc = ctx.enter_context(tile.TileContext(nc))

        # ---- persistent pools
        const = ctx.enter_context(tc.tile_pool(name="const", bufs=1))
        wpool = ctx.enter_context(tc.tile_pool(name="w", bufs=1))
        xpool = ctx.enter_context(tc.tile_pool(name="x", bufs=1))
        qk_pool = ctx.enter_context(tc.tile_pool(name="qk", bufs=1))
        vaug_pool = ctx.enter_context(tc.tile_pool(name="vaug", bufs=1))
        num_pool = ctx.enter_context(tc.tile_pool(name="num", bufs=1))

        onesb = const.tile([128, HPC], BF16)
        nc.scalar.dma_start(out=onesb, in_=ones8[:, :])
        maskb = const.tile([128, nkbk], F32)
        nc.scalar.dma_start(
            out=maskb, in_=maskb_d.rearrange("(j p) -> p j", p=128))

        # weights resident in SBUF
        wq_sb = wpool.tile([128, KC, CAT], BF16)
        wk_sb = wpool.tile([128, KC, CAT], BF16)
        wv_sb = wpool.tile([128, KC, CAT], BF16)
        wt_sb = wpool.tile([128, CAT // 128, D_MODEL], BF16)
        nc.scalar.dma_start(out=wv_sb, in_=wvT.rearrange("(kc p) c -> p kc c", p=128))
        nc.scalar.dma_start(out=wk_sb, in_=wkT.rearrange("(kc p) c -> p kc c", p=128))
        nc.scalar.dma_start(out=wq_sb, in_=wqT.rearrange("(kc p) c -> p kc c", p=128))
        nc.scalar.dma_start(out=wt_sb, in_=wtailT.rearrange("(c p) o -> p c o", p=128))

        # x (kept tokens first: k/v proj unblocks early; full x for q proj)
        xkts = [xpool.tile([128, tkp], BF16, tag=f"xk{kc}", name=f"xk{kc}")
                for kc in range(KC)]
        for kc in range(KC):
            nc.sync.dma_start(out=xkts[kc],
                              in_=xkT[kc * 128:(kc + 1) * 128, :])
        xts = [xpool.tile([128, T], BF16, tag=f"x{kc}", name=f"x{kc}")
               for kc in range(KC)]
        for half in range(2):
            for kc in range(KC):
                nc.sync.dma_start(
                    out=xts[kc][:, half * QH:(half + 1) * QH],
                    in_=xT[kc * 128:(kc + 1) * 128, half * QH:(half + 1) * QH])

        # persistent intermediates
        qts = [qk_pool.tile([128, T], BF16, tag=f"qt{j}", name=f"qt{j}")
               for j in range(NPAIR)]
        kts = [qk_pool.tile([128, tkp], BF16, tag=f"kt{j}", name=f"kt{j}")
               for j in range(NPAIR)]
        vaugs = [vaug_pool.tile([128, HPC, D_HEAD + 1], BF16, tag=f"va{t}",
                                name=f"va{t}") for t in range(nkbk)]
        nums = [num_pool.tile([128, T], BF16, tag=f"nm{j}", name=f"nm{j}")
                for j in range(NPAIR)]

        # ---- phase 1a: V projection (kept tokens), vps has all 8 PSUM banks
        with tc.tile_pool(name="vps", bufs=1, space="PSUM") as vps:
            done = 0
            while done < nkbk:
                g = min(8, nkbk - done)
                vp = [vps.tile([128, CAT], F32, tag=f"vp{i}", name=f"vp{i}")
                      for i in range(g)]
                for kc in range(KC):
                    for i in range(g):
                        nc.tensor.matmul(
                            vp[i],
                            xkts[kc][:, (done + i) * 128:(done + i + 1) * 128],
                            wv_sb[:, kc, :],
                            start=(kc == 0), stop=(kc == KC - 1),
                        )
                for i in range(g):
                    va = vaugs[done + i]
                    nc.scalar.dma_start(
                        out=va[:, :, D_HEAD:D_HEAD + 1], in_=ones8[:, :])
                    nc.scalar.copy(
                        out=va[:, :, 0:D_HEAD],
                        in_=vp[i].rearrange("p (h d) -> p h d", h=HPC),
                    )
                done += g

        # ---- k/q projection helpers (pps pool: [128,512] x2 = 2 PSUM banks)
        KCH = [(c * 512, min(512, tkp - c * 512)) for c in range((tkp + 511) // 512)]

        def _copy(eng, out, in_):
            if eng is nc.scalar:
                eng.copy(out=out, in_=in_)
            else:
                eng.tensor_copy(out=out, in_=in_)

        def emit_kproj_chunk(pps, j, c0, csz, eng):
            pp = pps.tile([128, QH], F32, tag="stp", name="pp")
            for kc in range(KC):
                nc.tensor.matmul(
                    pp[:, 0:csz],
                    wk_sb[:, kc, j * 128:(j + 1) * 128],
                    xkts[kc][:, c0:c0 + csz],
                    start=(kc == 0), stop=(kc == KC - 1),
                )
            _copy(eng, kts[j][:, c0:c0 + csz], pp[:, 0:csz])

        def emit_qproj_chunk(pps, j, c, eng):
            pp = pps.tile([128, QH], F32, tag="stp", name="pp")
            for kc in range(KC):
                nc.tensor.matmul(
                    pp[:, 0:512],
                    wq_sb[:, kc, j * 128:(j + 1) * 128],
                    xts[kc][:, c * 512:(c + 1) * 512],
                    start=(kc == 0), stop=(kc == KC - 1),
                )
            _copy(eng, qts[j][:, c * 512:(c + 1) * 512], pp[:, 0:512])

        def proj_chunks_for_pair(j, eng):
            # k first (attention consumes k of every block before q half 2)
            for c0, csz in KCH:
                yield lambda pps, j=j, c0=c0, csz=csz, eng=eng: \
                    emit_kproj_chunk(pps, j, c0, csz, eng)
            for c in range(T // 512):
                yield lambda pps, j=j, c=c, eng=eng: \
                    emit_qproj_chunk(pps, j, c, eng)

# ---- phase 2: attention; pair j+1 proj interleaved on the PE.
        # Projection chunks share the stps rotation (tag "stp") so avps can
        # double-buffer: normalize runs fully off the critical path.
        with tc.tile_pool(name="p_sb", bufs=5) as p_pool, \
             tc.tile_pool(name="r_sb", bufs=2) as r_pool, \
             tc.tile_pool(name="bc_sb", bufs=2) as bc_pool, \
             tc.tile_pool(name="stps", bufs=2, space="PSUM") as stps, \
             tc.tile_pool(name="avps", bufs=2, space="PSUM") as avps:
            pps = stps

            # pair-0 k/q projection up front (ACT idle: use it for copies)
            for emit in proj_chunks_for_pair(0, nc.scalar):
                emit(pps)

            def keeper(ptile):
                # tiny PE matmul to keep the tensor engine p-state warm
                nc.tensor.matmul(
                    ptile[0:HPC, 0:HPC], onesb, onesb,
                    start=True, stop=True, skip_group_check=True)

            for pair in range(NPAIR):
                proj_iter = iter(
                    proj_chunks_for_pair(pair + 1, nc.vector)
                    if pair + 1 < NPAIR else [])
                # spread the next pair's proj chunks over this pair's units
                per_unit = ((len(KCH) + T // 512) + 3) // 4

                for sub in range(2):
                    h = 2 * pair + sub
                    r0 = sub * 64
                    qt = qts[pair][r0:r0 + 64, :]
                    kt = kts[pair][r0:r0 + 64, :]
                    for half in range(2):
                        q0 = half * QH
                        avp = avps.tile([D_HEAD + 1, QH], F32, tag="avp",
                                        name="avp")
                        keeper(avp)
                        p_tiles = {}

                        def emit_st_exp(kb):
                            stp = stps.tile([128, QH], F32, tag="stp",
                                            name="stp")
                            if kb % 2 == 0:
                                keeper(stp)
                            for n in range(QH // 512):
                                nc.tensor.matmul(
                                    stp[:, n * 512:(n + 1) * 512],
                                    kt[:, kb * 128:(kb + 1) * 128],
                                    qt[:, q0 + n * 512:q0 + (n + 1) * 512],
                                    start=True, stop=True,
                                )
                            p_sb = p_pool.tile([128, QH], BF16, tag="p",
                                               name="p_sb")
                            nc.scalar.activation(
                                out=p_sb, in_=stp,
                                func=mybir.ActivationFunctionType.Exp,
                                bias=maskb[:, kb:kb + 1], scale=0.125,
                            )
                            p_tiles[kb] = p_sb

                        def emit_av(kb):
                            p_sb = p_tiles.pop(kb)
                            for n in range(QH // 512):
                                nc.tensor.matmul(
                                    avp[:, n * 512:(n + 1) * 512],
                                    vaugs[kb][:, h, :],
                                    p_sb[:, n * 512:(n + 1) * 512],
                                    start=(kb == 0), stop=(kb == nkbk - 1),
                                )

                        budget = per_unit
                        for kb in range(nkbk):
                            emit_st_exp(kb)
                            if kb >= LAG:
                                emit_av(kb - LAG)
                            elif budget > 0:
                                emit = next(proj_iter, None)
                                if emit is not None:
                                    emit(pps)
                                budget -= 1
                        for kb in range(max(0, nkbk - LAG), nkbk):
                            emit_av(kb)
                        while budget > 0:
                            emit = next(proj_iter, None)
                            if emit is None:
                                break
                            emit(pps)
                            budget -= 1

                        # normalize: per-token reciprocal of the ones-row,
                        # broadcast across the 64 cat partitions (gpsimd)
                        r_sb = r_pool.tile([1, QH], F32, tag="r", name="r_sb")
                        nc.vector.reciprocal(
                            out=r_sb, in_=avp[D_HEAD:D_HEAD + 1, :])
                        bc_sb = bc_pool.tile([D_HEAD, QH], F32, tag="bc",
                                             name="bc_sb")
                        nc.sync.dma_start(
                            out=bc_sb,
                            in_=r_sb[0:1, :].unsqueeze(1).to_broadcast(
                                (1, D_HEAD, QH)))
                        nc.vector.tensor_tensor(
                            out=nums[pair][r0:r0 + 64, q0:q0 + QH],
                            in0=avp[0:D_HEAD, :], in1=bc_sb,
                            op=mybir.AluOpType.mult,
                        )
                # drain any leftover proj chunks for the next pair
                for emit in proj_iter:
                    emit(pps)

        # ---- phase 3: tail matmul  y[tok, out] = attn_cat @ wtailT
        with tc.tile_pool(name="y_sb", bufs=3) as y_pool, \
             tc.tile_pool(name="yps", bufs=2, space="PSUM") as yps:
            for tb in range(T // 128):
                yp = yps.tile([128, D_MODEL], F32, tag="yp")
                keeper(yp)
                for n in range(D_MODEL // 512):
                    for c in range(CAT // 128):
                        nc.tensor.matmul(
                            yp[:, n * 512:(n + 1) * 512],
                            nums[c][:, tb * 128:(tb + 1) * 128],
                            wt_sb[:, c, n * 512:(n + 1) * 512],
                            start=(c == 0), stop=(c == CAT // 128 - 1),
                        )
                y_sb = y_pool.tile([128, D_MODEL], F32, tag="ys")
                nc.scalar.copy(out=y_sb, in_=yp)
                nc.sync.dma_start(out=y[tb * 128:(tb + 1) * 128, :], in_=y_sb)

    if split_waits:
        split_excess_waits(nc)
    return nc


_NC_CACHE = {}


def _get_nc(tkp):
    if tkp not in _NC_CACHE:
        _NC_CACHE[tkp] = build_nc(tkp)
    return _NC_CACHE[tkp]


def make_in_maps(x, mask, w_qkv, w_tail, tkp):
    """Shard full inputs into 8 per-core input maps (mask-compacted)."""
    import ml_dtypes
    bf16 = ml_dtypes.bfloat16
    x = np.asarray(x, dtype=np.float32)
    mask = np.asarray(mask, dtype=np.int32)
    w_qkv = np.asarray(w_qkv, dtype=np.float32)
    w_tail = np.asarray(w_tail, dtype=np.float32)

    w3 = w_qkv.reshape(N_HEAD, 3, D_HEAD, D_MODEL)  # [head, q|k|v, d, dmodel]
    in_maps = []
    for c in range(8):
        b, hg = c // 2, c % 2
        heads = list(range(hg * HPC, (hg + 1) * HPC))
        kept = np.nonzero(mask[b])[0]
        tk = len(kept)
        assert tk <= tkp
        # compacted x for k/v projections, zero-padded to tkp
        xk = np.zeros((tkp, D_MODEL), dtype=np.float32)
        xk[:tk] = x[b][kept]
        maskb = np.full((tkp,), NEG, dtype=np.float32)
        maskb[:tk] = 0.0
        # per-pair packed q/k weights: cols j*128+(0:64)=head 2j, (64:128)=2j+1
        wq = np.concatenate([w3[h, 0] for h in heads], axis=0)  # [512, 1024]
        wk = np.concatenate([w3[h, 1] for h in heads], axis=0)
        wv = np.concatenate([w3[h, 2] for h in heads], axis=0)
        wt = w_tail[:, hg * CAT:(hg + 1) * CAT]  # [1024, 512]
        in_maps.append({
            "ones8": np.ones((128, HPC), dtype=bf16),
            "xT": np.ascontiguousarray(x[b].T).astype(bf16),
            "xkT": np.ascontiguousarray(xk.T).astype(bf16),
            "wqT": np.ascontiguousarray(wq.T).astype(bf16),
            "wkT": np.ascontiguousarray(wk.T).astype(bf16),
            "wvT": np.ascontiguousarray(wv.T).astype(bf16),
            "wtailT": np.ascontiguousarray(wt.T).astype(bf16),
            "maskb": maskb,
        })
    return in_maps


def _tkp_for(mask):
    mask = np.asarray(mask)
    mx = max(int((mask[b] != 0).sum()) for b in range(mask.shape[0]))
    return max(128, ((mx + 127) // 128) * 128)


def kernel(x, mask, w_qkv, w_tail, b_tail):
    tkp = _tkp_for(mask)
    nc = _get_nc(tkp)
    in_maps = make_in_maps(x, mask, w_qkv, w_tail, tkp)
    last_err = None
    for _attempt in range(3):
        try:
            res = run_bass_kernel_spmd(nc, in_maps, list(range(8))).results
            break
        except Exception as e:  # transient device/runtime errors: retry
            last_err = e
            _time.sleep(3.0)
    else:
        raise last_err
    out = np.empty((BN, T, D_MODEL), dtype=np.float32)
    b_tail = np.asarray(b_tail, dtype=np.float32)
    for b in range(BN):
        out[b] = res[2 * b]["y"] + res[2 * b + 1]["y"] + b_tail
    return out


# revision 13
# speedup vs baseline: 1.1938x; 1.1938x over previous
"""Multi-head attention Trainium2 kernel, 8-way sharded, mask-compacted.

Problem: x[4,2048,1024] -> qkv proj (w_qkv [3072,1024]) -> 16-head attention
with key-padding mask -> tail proj (w_tail [1024,1024]) + b_tail.

Sharding: 8 shards = 4 batches x 2 head-groups (8 heads each). Each core
computes, for its (batch b, head-group hg):
  - q projection of x[b] for its 8 heads (all 2048 tokens)
  - k/v projections of the MASK-KEPT tokens only (host-compacted; masked
    keys contribute exactly zero to the softmax, so dropping them up front
    is exact and halves the attention work)
  - [kept x 2048] masked attention per head
  - partial tail matmul y_part = attn_cat @ w_tail[:, cat_slice].T
Host unshards: out[b] = y_part[2b] + y_part[2b+1] + b_tail.  No collectives.

All matmul operands are bf16 (PE 1 cyc/row); PSUM accumulation is f32.
Softmax denominator comes from a ones-column appended to V; the per-token
reciprocal is broadcast across cat-partitions via gpsimd partition_broadcast
(no PE transposes). q/k projections of pair j+1 are interleaved into pair
j's attention stream to keep the PE dense (p-state) while ACT runs exp.
"""

import time as _time

import numpy as np
from contextlib import ExitStack

import concourse.bass as bass
import concourse.mybir as mybir
import concourse.tile as tile
from concourse.bass_utils import run_bass_kernel_spmd

# ---------------------------------------------------------------------------
# walrus in this env accepts at most 2 sync waits per instruction; Tile's
# scheduler emits up to 10. Post-pass: peel excess waits onto same-engine
# NoOps inserted immediately before the offending instruction (same engine
# stream position => identical synchronization semantics).
MAX_WAITS = 1


def split_excess_waits(nc):
    for fn in nc.m.functions:
        for bb in fn.blocks:
            insts = list(bb.instructions)
            out = []
            changed = False
            for inst in insts:
                si = inst.sync_info
                waits = list(si.on_wait) if si is not None else []
                if len(waits) > MAX_WAITS:
                    extra = waits[:-MAX_WAITS]
                    for ci in range(0, len(extra), MAX_WAITS):
                        chunk = extra[ci:ci + MAX_WAITS]
                        nop = mybir.InstNoOp(
                            name=f"{inst.name}-ws{ci}", ins=[], outs=[])
                        nop.engine = inst.engine
                        nop.sync_info = mybir.SyncInfo(
                            on_wait=chunk, on_update=[])
                        out.append(nop)
                    inst.sync_info = mybir.SyncInfo(
                        on_wait=waits[-MAX_WAITS:],
                        on_update=list(si.on_update))
                    changed = True
                out.append(inst)
            if changed:
                bb.instructions = out
# ---------------------------------------------------------------------------

D_MODEL = 1024
N_HEAD = 16
D_HEAD = 64
BN, T = 4, 2048
HPC = 8                      # heads per core
NPAIR = HPC // 2             # head pairs (2 heads share a 128-row tile)
CAT = HPC * D_HEAD           # 512 per-core tail contraction
QH = T // 2                  # q processed in two halves of 1024
KC = D_MODEL // 128          # 8 contraction chunks
F32 = mybir.dt.float32
BF16 = mybir.dt.bfloat16
LAG = 4                      # ST->AV software pipeline depth (key blocks)
NEG = -30000.0               # additive bias for padded keys: exp -> 0


def build_nc(tkp, split_waits=True):
    """tkp: padded kept-key count (multiple of 128)."""
    nkbk = tkp // 128        # kept-key blocks
    nc = bass.Bass()
    xT = nc.declare_dram_parameter("xT", [D_MODEL, T], BF16, isOutput=False)
    xkT = nc.declare_dram_parameter("xkT", [D_MODEL, tkp], BF16, isOutput=False)
    wqT = nc.declare_dram_parameter("wqT", [D_MODEL, CAT], BF16, isOutput=False)
    wkT = nc.declare_dram_parameter("wkT", [D_MODEL, CAT], BF16, isOutput=False)
    wvT = nc.declare_dram_parameter("wvT", [D_MODEL, CAT], BF16, isOutput=False)
    wtailT = nc.declare_dram_parameter("wtailT", [CAT, D_MODEL], BF16, isOutput=False)
    maskb_d = nc.declare_dram_parameter("maskb", [tkp], F32, isOutput=False)
    ones8 = nc.declare_dram_parameter("ones8", [128, HPC], BF16, isOutput=False)
    y = nc.declare_dram_parameter("y", [T, D_MODEL], F32, isOutput=True)

    with ExitStack() as ctx:
        tc = ctx.enter_context(tile.TileContext(nc))

        # ---- persistent pools
        const = ctx.enter_context(tc.tile_pool(name="const", bufs=1))
        wpool = ctx.enter_context(tc.tile_pool(name="w", bufs=1))
        xpool = ctx.enter_context(tc.tile_pool(name="x", bufs=1))
        qk_pool = ctx.enter_context(tc.tile_pool(name="qk", bufs=1))
        vaug_pool = ctx.enter_context(tc.tile_pool(name="vaug", bufs=1))
        num_pool = ctx.enter_context(tc.tile_pool(name="num", bufs=1))

        onesb = const.tile([128, HPC], BF16)
        nc.scalar.dma_start(out=onesb, in_=ones8[:, :])
        maskb = const.tile([128, nkbk], F32)
        nc.scalar.dma_start(
            out=maskb, in_=maskb_d.rearrange("(j p) -> p j", p=128))

        # weights resident in SBUF
        wq_sb = wpool.tile([128, KC, CAT], BF16)
        wk_sb = wpool.tile([128, KC, CAT], BF16)
        wv_sb = wpool.tile([128, KC, CAT], BF16)
        wt_sb = wpool.tile([128, CAT // 128, D_MODEL], BF16)
        nc.scalar.dma_start(out=wv_sb, in_=wvT.rearrange("(kc p) c -> p kc c", p=128))
        nc.scalar.dma_start(out=wk_sb, in_=wkT.rearrange("(kc p) c -> p kc c", p=128))
        nc.scalar.dma_start(out=wq_sb, in_=wqT.rearrange("(kc p) c -> p kc c", p=128))
        nc.scalar.dma_start(out=wt_sb, in_=wtailT.rearrange("(c p) o -> p c o", p=128))

        # x (kept tokens first: k/v proj unblocks early; full x for q proj)
        xkts = [xpool.tile([128, tkp], BF16, tag=f"xk{kc}", name=f"xk{kc}")
                for kc in range(KC)]
        for kc in range(KC):
            nc.sync.dma_start(out=xkts[kc],
                              in_=xkT[kc * 128:(kc + 1) * 128, :])
        xts = [xpool.tile([128, T], BF16, tag=f"x{kc}", name=f"x{kc}")
               for kc in range(KC)]
        for half in range(2):
            for kc in range(KC):
                nc.sync.dma_start(
                    out=xts[kc][:, half * QH:(half + 1) * QH],
                    in_=xT[kc * 128:(kc + 1) * 128, half * QH:(half + 1) * QH])

        # persistent intermediates
        qts = [qk_pool.tile([128, T], BF16, tag=f"qt{j}", name=f"qt{j}")
               for j in range(NPAIR)]
        kts = [qk_pool.tile([128, tkp], BF16, tag=f"kt{j}", name=f"kt{j}")
               for j in range(NPAIR)]
        vaugs = [vaug_pool.tile([128, HPC, D_HEAD + 1], BF16, tag=f"va{t}",
                                name=f"va{t}") for t in range(nkbk)]
        nums = [num_pool.tile([128, T], BF16, tag=f"nm{j}", name=f"nm{j}")
                for j in range(NPAIR)]

        # ---- phase 1a: V projection (kept tokens), vps has all 8 PSUM banks
        with tc.tile_pool(name="vps", bufs=1, space="PSUM") as vps:
            done = 0
            while done < nkbk:
                g = min(8, nkbk - done)
                vp = [vps.tile([128, CAT], F32, tag=f"vp{i}", name=f"vp{i}")
                      for i in range(g)]
                for kc in range(KC):
                    for i in range(g):
                        nc.tensor.matmul(
                            vp[i],
                            xkts[kc][:, (done + i) * 128:(done + i + 1) * 128],
                            wv_sb[:, kc, :],
                            start=(kc == 0), stop=(kc == KC - 1),
                        )
                for i in range(g):
                    va = vaugs[done + i]
                    nc.scalar.dma_start(
                        out=va[:, :, D_HEAD:D_HEAD + 1], in_=ones8[:, :])
                    nc.scalar.copy(
                        out=va[:, :, 0:D_HEAD],
                        in_=vp[i].rearrange("p (h d) -> p h d", h=HPC),
                    )
                done += g

        # ---- k/q projection helpers (pps pool: [128,512] x2 = 2 PSUM banks)
        KCH = [(c * 512, min(512, tkp - c * 512)) for c in range((tkp + 511) // 512)]

        def _copy(eng, out, in_):
            if eng is nc.scalar:
                eng.copy(out=out, in_=in_)
            else:
                eng.tensor_copy(out=out, in_=in_)

        def emit_kproj_chunk(pps, j, c0, csz, eng):
            pp = pps.tile([128, QH], F32, tag="stp", name="pp")
            for kc in range(KC):
                nc.tensor.matmul(
                    pp[:, 0:csz],
                    wk_sb[:, kc, j * 128:(j + 1) * 128],
                    xkts[kc][:, c0:c0 + csz],
                    start=(kc == 0), stop=(kc == KC - 1),
                )
            _copy(eng, kts[j][:, c0:c0 + csz], pp[:, 0:csz])

        def emit_qproj_chunk(pps, j, c, eng):
            pp = pps.tile([128, QH], F32, tag="stp", name="pp")
            for kc in range(KC):
                nc.tensor.matmul(
                    pp[:, 0:512],
                    wq_sb[:, kc, j * 128:(j + 1) * 128],
                    xts[kc][:, c * 512:(c + 1) * 512],
                    start=(kc == 0), stop=(kc == KC - 1),
                )
            _copy(eng, qts[j][:, c * 512:(c + 1) * 512], pp[:, 0:512])

        def proj_chunks_for_pair(j, eng):
            # k first (attention consumes k of every block before q half 2)
            for c0, csz in KCH:
                yield lambda pps, j=j, c0=c0, csz=csz, eng=eng: \
                    emit_kproj_chunk(pps, j, c0, csz, eng)
            for c in range(T // 512):
                yield lambda pps, j=j, c=c, eng=eng: \
                    emit_qproj_chunk(pps, j, c, eng)

# ---- phase 2: attention; pair j+1 proj interleaved on the PE.
        # Projection chunks share the stps rotation (tag "stp") so avps can
        # double-buffer: normalize runs fully off the critical path.
        with tc.tile_pool(name="p_sb", bufs=5) as p_pool, \
             tc.tile_pool(name="r_sb", bufs=2) as r_pool, \
             tc.tile_pool(name="bc_sb", bufs=2) as bc_pool, \
             tc.tile_pool(name="stps", bufs=2, space="PSUM") as stps, \
             tc.tile_pool(name="avps", bufs=2, space="PSUM") as avps:
            pps = stps

            # pair-0 k/q projection up front (ACT idle: use it for copies)
            for emit in proj_chunks_for_pair(0, nc.scalar):
                emit(pps)

            def keeper(ptile):
                # tiny PE matmul to keep the tensor engine p-state warm
                nc.tensor.matmul(
                    ptile[0:HPC, 0:HPC], onesb, onesb,
                    start=True, stop=True, skip_group_check=True)

            def emit_norm(avp, pair, r0, q0):
                # r = exp(-ln(D)) on ACT: Ln/Exp/Copy share one act table
                # set, and ACT has slack; the DVE reciprocal is 6.5us/row.
                t_sb = r_pool.tile([1, QH], F32, tag="t", name="t_sb")
                nc.scalar.activation(
                    out=t_sb, in_=avp[D_HEAD:D_HEAD + 1, :],
                    func=mybir.ActivationFunctionType.Ln)
                r_sb = r_pool.tile([1, QH], F32, tag="r", name="r_sb")
                nc.scalar.activation(
                    out=r_sb, in_=t_sb,
                    func=mybir.ActivationFunctionType.Exp, scale=-1.0)
                bc_sb = bc_pool.tile([D_HEAD, QH], F32, tag="bc",
                                     name="bc_sb")
                nc.sync.dma_start(
                    out=bc_sb,
                    in_=r_sb[0:1, :].unsqueeze(1).to_broadcast(
                        (1, D_HEAD, QH)))
                nc.vector.tensor_tensor(
                    out=nums[pair][r0:r0 + 64, q0:q0 + QH],
                    in0=avp[0:D_HEAD, :], in1=bc_sb,
                    op=mybir.AluOpType.mult,
                )

            pending_norm = None
            for pair in range(NPAIR):
                proj_iter = iter(
                    proj_chunks_for_pair(pair + 1, nc.vector)
                    if pair + 1 < NPAIR else [])
                # spread the next pair's proj chunks over this pair's units
                per_unit = ((len(KCH) + T // 512) + 3) // 4

                for sub in range(2):
                    h = 2 * pair + sub
                    r0 = sub * 64
                    qt = qts[pair][r0:r0 + 64, :]
                    kt = kts[pair][r0:r0 + 64, :]
                    for half in range(2):
                        q0 = half * QH
                        avp = avps.tile([D_HEAD + 1, QH], F32, tag="avp",
                                        name="avp")
                        keeper(avp)
                        p_tiles = {}

                        def emit_st_exp(kb):
                            stp = stps.tile([128, QH], F32, tag="stp",
                                            name="stp")
                            if kb % 2 == 0:
                                keeper(stp)
                            for n in range(QH // 512):
                                nc.tensor.matmul(
                                    stp[:, n * 512:(n + 1) * 512],
                                    kt[:, kb * 128:(kb + 1) * 128],
                                    qt[:, q0 + n * 512:q0 + (n + 1) * 512],
                                    start=True, stop=True,
                                )
                            p_sb = p_pool.tile([128, QH], BF16, tag="p",
                                               name="p_sb")
                            nc.scalar.activation(
                                out=p_sb, in_=stp,
                                func=mybir.ActivationFunctionType.Exp,
                                bias=maskb[:, kb:kb + 1], scale=0.125,
                            )
                            p_tiles[kb] = p_sb

                        def emit_av(kb):
                            p_sb = p_tiles.pop(kb)
                            for n in range(QH // 512):
                                nc.tensor.matmul(
                                    avp[:, n * 512:(n + 1) * 512],
                                    vaugs[kb][:, h, :],
                                    p_sb[:, n * 512:(n + 1) * 512],
                                    start=(kb == 0), stop=(kb == nkbk - 1),
                                )

                        budget = per_unit
                        for kb in range(nkbk):
                            emit_st_exp(kb)
                            if kb >= LAG:
                                emit_av(kb - LAG)
                            else:
                                if budget > 0:
                                    emit = next(proj_iter, None)
                                    if emit is not None:
                                        emit(pps)
                                    budget -= 1
                                if kb == LAG - 1 and pending_norm is not None:
                                    # previous unit's normalize, emitted after
                                    # this unit's proj copies so the DVE
                                    # stream doesn't block the stp rotation
                                    emit_norm(*pending_norm)
                                    pending_norm = None
                        for kb in range(max(0, nkbk - LAG), nkbk):
                            emit_av(kb)
                        while budget > 0:
                            emit = next(proj_iter, None)
                            if emit is None:
                                break
                            emit(pps)
                            budget -= 1
                        pending_norm = (avp, pair, r0, q0)
                # drain any leftover proj chunks for the next pair
                for emit in proj_iter:
                    emit(pps)
            if pending_norm is not None:
                emit_norm(*pending_norm)

        # ---- phase 3: tail matmul  y[tok, out] = attn_cat @ wtailT
        with tc.tile_pool(name="y_sb", bufs=3) as y_pool, \
             tc.tile_pool(name="yps", bufs=2, space="PSUM") as yps:
            for tb in range(T // 128):
                yp = yps.tile([128, D_MODEL], F32, tag="yp")
                keeper(yp)
                for n in range(D_MODEL // 512):
                    for c in range(CAT // 128):
                        nc.tensor.matmul(
                            yp[:, n * 512:(n + 1) * 512],
                            nums[c][:, tb * 128:(tb + 1) * 128],
                            wt_sb[:, c, n * 512:(n + 1) * 512],
                            start=(c == 0), stop=(c == CAT // 128 - 1),
                        )
                y_sb = y_pool.tile([128, D_MODEL], F32, tag="ys")
                nc.scalar.copy(out=y_sb, in_=yp)
                nc.sync.dma_start(out=y[tb * 128:(tb + 1) * 128, :], in_=y_sb)

    if split_waits:
        split_excess_waits(nc)
    return nc


_NC_CACHE = {}


def _get_nc(tkp):
    if tkp not in _NC_CACHE:
        _NC_CACHE[tkp] = build_nc(tkp)
    return _NC_CACHE[tkp]


def make_in_maps(x, mask, w_qkv, w_tail, tkp):
    """Shard full inputs into 8 per-core input maps (mask-compacted)."""
    import ml_dtypes
    bf16 = ml_dtypes.bfloat16
    x = np.asarray(x, dtype=np.float32)
    mask = np.asarray(mask, dtype=np.int32)
    w_qkv = np.asarray(w_qkv, dtype=np.float32)
    w_tail = np.asarray(w_tail, dtype=np.float32)

    w3 = w_qkv.reshape(N_HEAD, 3, D_HEAD, D_MODEL)  # [head, q|k|v, d, dmodel]
    in_maps = []
    for c in range(8):
        b, hg = c // 2, c % 2
        heads = list(range(hg * HPC, (hg + 1) * HPC))
        kept = np.nonzero(mask[b])[0]
        tk = len(kept)
        assert tk <= tkp
        # compacted x for k/v projections, zero-padded to tkp
        xk = np.zeros((tkp, D_MODEL), dtype=np.float32)
        xk[:tk] = x[b][kept]
        maskb = np.full((tkp,), NEG, dtype=np.float32)
        maskb[:tk] = 0.0
        # per-pair packed q/k weights: cols j*128+(0:64)=head 2j, (64:128)=2j+1
        wq = np.concatenate([w3[h, 0] for h in heads], axis=0)  # [512, 1024]
        wk = np.concatenate([w3[h, 1] for h in heads], axis=0)
        wv = np.concatenate([w3[h, 2] for h in heads], axis=0)
        wt = w_tail[:, hg * CAT:(hg + 1) * CAT]  # [1024, 512]
        in_maps.append({
            "ones8": np.ones((128, HPC), dtype=bf16),
            "xT": np.ascontiguousarray(x[b].T).astype(bf16),
            "xkT": np.ascontiguousarray(xk.T).astype(bf16),
            "wqT": np.ascontiguousarray(wq.T).astype(bf16),
            "wkT": np.ascontiguousarray(wk.T).astype(bf16),
            "wvT": np.ascontiguousarray(wv.T).astype(bf16),
            "wtailT": np.ascontiguousarray(wt.T).astype(bf16),
            "maskb": maskb,
        })
    return in_maps


def _tkp_for(mask):
    mask = np.asarray(mask)
    mx = max(int((mask[b] != 0).sum()) for b in range(mask.shape[0]))
    return max(128, ((mx + 127) // 128) * 128)


def kernel(x, mask, w_qkv, w_tail, b_tail):
    tkp = _tkp_for(mask)
    nc = _get_nc(tkp)
    in_maps = make_in_maps(x, mask, w_qkv, w_tail, tkp)
    last_err = None
    for _attempt in range(3):
        try:
            res = run_bass_kernel_spmd(nc, in_maps, list(range(8))).results
            break
        except Exception as e:  # transient device/runtime errors: retry
            last_err = e
            _time.sleep(3.0)
    else:
        raise last_err
    out = np.empty((BN, T, D_MODEL), dtype=np.float32)
    b_tail = np.asarray(b_tail, dtype=np.float32)
    for b in range(BN):
        out[b] = res[2 * b]["y"] + res[2 * b + 1]["y"] + b_tail
    return out
